# revision 9
# baseline (speedup 1.0000x reference)
"""Trainium2 Bass kernel for nn_ChannelSelfAttention.

Reference computation (per batch sample b):
    xt   = x[b].T                          # [C, L]
    q    = xt @ Wq.T + bq                  # [C, H]
    kv   = xt @ Wkv.T + bkv                # [C, 2H] -> k, v
    attn = (q * H**-0.5) @ k.T             # [C, C]  (no softmax)
    y    = attn @ v                        # [C, H]
    g    = mean(y, axis=-1)                # [C]
    out[b] = x[b] * g[None, :]             # [L, C]

There is no softmax, so the mean over H commutes through the matmuls:
    g = attn @ vbar            with vbar = mean_h v  (a [C] vector)
      = q_s @ (k^T @ vbar)     with kbar = k^T vbar  (an [H] vector)
and vbar only needs the folded weight row wvbar = mean_h Wv, which the
host replicates 64x so the projection matmul itself broadcasts vbar
onto 64 partitions (psum_k = [k^T ; vbar x64], [128, C]). The [C,C]
attention is never materialized.

Sharding: data-parallel over B across 8 cores (4 samples per core);
weights replicated. All DRAM I/O is fp16 (memory-bound problem): the
host casts x/W to fp16 (attn scale 1/8 folded into Wq exactly), the
kernel writes fp16 output, host widens to f32. L-contractions
accumulate in f32 PSUM; the post-projection stage is f32.

Layout: l = p*32 + m, i.e. x[b] reshaped [128, 32, 256] — partition-
contiguous descriptors, no host transpose for input or output.

Software pipelining: each sample's tiny stage-2 (kbar reduce, g
matmul) is emitted interleaved into the NEXT sample's projection
matmul stream, so the tensor engine never stalls on Act/DVE hops and
keeps its ramped p-state.
"""

import numpy as np

import concourse.bass as bass
import concourse.mybir as mybir
import concourse.tile as tile
from concourse import bacc
from concourse.bass_utils import run_bass_kernel_spmd

B, L, C, H = 32, 4096, 256, 64
N_CORES = 8
B_LOC = B // N_CORES          # samples per core
P = 128                       # SBUF partitions
M = L // P                    # l-rows per partition (l = p*M + m)
F16 = mybir.dt.float16
F32 = mybir.dt.float32
F32R = mybir.dt.float32r
SCALE = float(H) ** -0.5      # exactly 1/8; folded into Wq on host
IDENT = mybir.ActivationFunctionType.Identity


def _emit(tc: "tile.TileContext", x_d, wT_d, bq_d, bk_d, out_d):
    nc = tc.nc
    QT = M // 4                                      # quarter = 8 m-chunks
    with (
        tc.tile_pool(name="singles", bufs=1) as singles,
        tc.tile_pool(name="xin", bufs=4) as xin,
        tc.tile_pool(name="xout", bufs=3) as xout,
        tc.tile_pool(name="small", bufs=2) as small,
        tc.tile_pool(name="psum", bufs=2, space="PSUM") as psum,
    ):
        # ---- one-time loads (scalar ring; x loads ride the sync ring).
        # First weight chunk separate so sample 0 doesn't gate on 1.5 MiB.
        wT_sb = singles.tile([P, M, 3 * H], F16)     # 1.5 MiB
        nc.scalar.dma_start(out=wT_sb[:, 0:1], in_=wT_d[:, 0:1])
        nc.scalar.dma_start(out=wT_sb[:, 1:M], in_=wT_d[:, 1:M])
        bq_sb = singles.tile([H, 1], F32)            # already scaled by 1/8
        nc.scalar.dma_start(out=bq_sb, in_=bq_d[:].rearrange("(h o) -> h o", o=1))
        bk_sb = singles.tile([P, 1], F32)            # [k bias ; mean(v bias) x64]
        nc.scalar.dma_start(out=bk_sb, in_=bk_d[:].rearrange("(h o) -> h o", o=1))

        S = [dict() for _ in range(B_LOC)]

        def load(b):
            x_sb = xin.tile([P, M * C], F16, tag="x")
            S[b]["x"] = x_sb
            for qt in range(4):
                sl = slice(qt * QT * C, (qt + 1) * QT * C)
                nc.sync.dma_start(out=x_sb[:, sl], in_=x_d[b][:, sl])

        def proj(b, which, m0, m1):
            # 'q': wT cols 0:64 -> q^T [64,C]
            # 'k': wT cols 64:192 -> [k^T ; vbar-broadcast] [128,C]
            if which == "q":
                if m0 == 0:
                    S[b]["pq"] = psum.tile([H, C], F32, tag="q", name="pq")
                pt, c0, c1 = S[b]["pq"], 0, H
            else:
                if m0 == 0:
                    S[b]["pk"] = psum.tile([P, C], F32, tag="k", name="pk")
                pt, c0, c1 = S[b]["pk"], H, 3 * H
            x_sb = S[b]["x"]
            for m in range(m0, m1):
                nc.tensor.matmul(
                    pt,
                    lhsT=wT_sb[:, m, c0:c1],
                    rhs=x_sb[:, m * C : (m + 1) * C],
                    start=(m == 0),
                    stop=(m == M - 1),
                )

        def aqk(b):
            q_sb = small.tile([H, C], F32R, tag="q_sb")
            nc.scalar.activation(q_sb[:], S[b]["pq"], IDENT, bias=bq_sb[:])
            ks_sb = small.tile([P, C], F32R, tag="ks_sb")
            nc.scalar.activation(ks_sb[:], S[b]["pk"], IDENT, bias=bk_sb[:])
            S[b]["q"], S[b]["ks"] = q_sb, ks_sb

        def vb(b):
            # realign the vbar broadcast (partitions 64:128) to base 0 via
            # an SBUF->SBUF DMA on the otherwise idle vector ring
            vb_sb = small.tile([H, C], F32, tag="vb_sb")
            nc.gpsimd.dma_start(out=vb_sb[:], in_=S[b]["ks"][H:P, :].bitcast(F32))
            S[b]["vb"] = vb_sb

        def ttr(b):
            # kbar[h] = sum_c k^T[h,c] * vbar[c]  (DVE, fused mult+sum)
            scr = small.tile([H, C], F32, tag="scr")
            kbar = small.tile([H, 1], F32, tag="kbar")
            nc.vector.scalar_tensor_tensor(
                out=scr[:],
                in0=S[b]["ks"][0:H, :].bitcast(F32),
                scalar=1.0,
                in1=S[b]["vb"][:],
                op0=mybir.AluOpType.mult,
                op1=mybir.AluOpType.mult,
                accum_out=kbar[:],
            )
            S[b]["kbar"] = kbar

        def kbc(b):
            # broadcast kbar [64,1] along free dim -> [64,128] (in*0 + bias)
            kb_bc = small.tile([H, P], F32R, tag="kb_bc")
            nc.scalar.activation(
                kb_bc[:], S[b]["q"][:, 0:P], IDENT, bias=S[b]["kbar"][:], scale=0.0
            )
            S[b]["kb_bc"] = kb_bc

        def g(b):
            # g[c] = sum_h q^T[h,c] kbar[h], broadcast to all 128 partitions
            pg = psum.tile([P, C], F32, tag="g")
            nc.tensor.matmul(
                pg, lhsT=S[b]["kb_bc"][:], rhs=S[b]["q"][:], skip_group_check=True
            )
            S[b]["pg"] = pg

        def g16(b):
            t = small.tile([P, C], F16, tag="g16")
            nc.scalar.copy(t, S[b]["pg"])
            S[b]["g16"] = t

        def gate_store(b):
            out_sb = xout.tile([P, M * C], F16, tag="out")
            gt = S[b]["g16"]
            g_bc = bass.AP(
                tensor=gt.tensor,
                offset=gt.offset,
                ap=[list(gt.ap[0]), [0, QT], list(gt.ap[1])],
            )
            for qt in range(4):
                sl = slice(qt * QT * C, (qt + 1) * QT * C)
                nc.vector.tensor_tensor(
                    out=out_sb[:, sl].rearrange("p (m c) -> p m c", c=C),
                    in0=S[b]["x"][:, sl].rearrange("p (m c) -> p m c", c=C),
                    in1=g_bc,
                    op=mybir.AluOpType.mult,
                )
                nc.scalar.dma_start(out=out_d[b][:, sl], in_=out_sb[:, sl])

        # ---- schedule: stage-2 of sample b rides inside sample b+1's
        # projection stream so the PE never waits on Act/DVE. ----
        for b in range(B_LOC):
            load(b)
        proj(0, "q", 0, M)
        proj(0, "k", 0, M)
        aqk(0)
        vb(0)
        for b in range(B_LOC):
            nb = b + 1
            if nb < B_LOC:
                # ~6 chunks of PE filler cover the aqk->vb->ttr->kbc latency,
                # so g(b) hits the PE with its deps already satisfied.
                proj(nb, "q", 0, 6)
                ttr(b)
                kbc(b)
                g(b)
                proj(nb, "q", 6, M)
                g16(b)
                gate_store(b)
                proj(nb, "k", 0, M)
                aqk(nb)
                vb(nb)
            else:
                ttr(b)
                kbc(b)
                g(b)
                g16(b)
                gate_store(b)


def build():
    nc = bacc.Bacc(
        "TRN2", target_bir_lowering=False, debug=False, num_devices=N_CORES
    )
    x_d = nc.dram_tensor("x", [B_LOC, P, M * C], F16, kind="ExternalInput")
    wT_d = nc.dram_tensor("wT", [P, M, 3 * H], F16, kind="ExternalInput")
    bq_d = nc.dram_tensor("bq", [H], F32, kind="ExternalInput")
    bk_d = nc.dram_tensor("bk", [P], F32, kind="ExternalInput")
    out_d = nc.dram_tensor("out", [B_LOC, P, M * C], F16, kind="ExternalOutput")
    with tile.TileContext(nc) as tc:
        _emit(tc, x_d, wT_d, bq_d, bk_d, out_d)
    nc.compile()
    return nc


_nc_cache = None


def _get_nc():
    global _nc_cache
    if _nc_cache is None:
        _nc_cache = build()
    return _nc_cache


def make_in_maps(x, Wq, bq, Wkv, bkv):
    x16 = np.asarray(x, dtype=np.float32).astype(np.float16)
    Wq32 = np.asarray(Wq, np.float32)
    Wkv32 = np.asarray(Wkv, np.float32)
    wvbar = Wkv32[H : 2 * H].mean(axis=0)            # folded mean_h Wv, [L]
    w192 = np.concatenate(
        [Wq32 * SCALE, Wkv32[0:H], np.tile(wvbar, (H, 1))], axis=0
    )
    wT = np.ascontiguousarray(w192.T.astype(np.float16).reshape(P, M, 3 * H))
    bq_s = np.ascontiguousarray(np.asarray(bq, np.float32) * SCALE)
    bkv32 = np.asarray(bkv, np.float32)
    bk128 = np.ascontiguousarray(
        np.concatenate(
            [bkv32[0:H], np.full(H, bkv32[H:].mean())]
        ).astype(np.float32)
    )
    return [
        {
            "x": x16[i * B_LOC : (i + 1) * B_LOC].reshape(B_LOC, P, M * C),
            "wT": wT,
            "bq": bq_s,
            "bk": bk128,
        }
        for i in range(N_CORES)
    ]


def run(inputs, **spmd_kwargs):
    """Run on hardware; returns (full_output, BassKernelResults)."""
    nc = _get_nc()
    in_maps = make_in_maps(**inputs)
    res = run_bass_kernel_spmd(nc, in_maps, list(range(N_CORES)), **spmd_kwargs)
    out = np.concatenate(
        [np.asarray(r["out"]).reshape(B_LOC, L, C) for r in res.results], axis=0
    ).astype(np.float32)
    return out, res


def kernel(**inputs) -> np.ndarray:
    out, _ = run(inputs)
    return out
